# revision 1
# baseline (speedup 1.0000x reference)
"""Causal self-attention (QKV projection + softmax(QK^T/sqrt(N)) @ V) on 8 TRN2
NeuronCores.

Sharding: core c = 2*b + j handles batch element b (of 4) and half its query
rows as four 256-row blocks, interleaved for causal load balance:
  j=0 -> blocks [7,4,3,0], j=1 -> blocks [6,5,2,1]   (block i = rows 256i..256i+255)
Block i needs k-tiles 0..2i+1 (2i+2 of them).  The SPMD schedule runs four
query slots with fixed k-tile counts KS=[16,12,8,4] (the per-rank max over
both cores' sorted needs); per-core causal masks (from shipped position
vectors) zero invalid/extra tiles.  Only the last 4 k-tiles of each slot can
ever be partial/invalid, so exactly those are masked.

Everything is bf16 (rel-err budget is 2e-2; bf16 lands ~2e-3): halves DMA
traffic and SBUF footprint so both K^T and V stay SBUF-resident for the whole
kernel -- no DRAM staging roundtrip -- and enables FWL fast weight loads.

Layout trick (from the fp32r ancestor): context ships pre-transposed [D, N] so
Q^T / K^T come out of the projection directly as [e, n] tiles and V as
[n, e]; scores are computed transposed S^T[k, q] = K^T.T @ Q^T, softmax runs
without max-subtraction (logits are O(1)), the per-query denominator comes
from a ones-vector matmul, and the exp'd S^T is exactly the lhsT that PV
needs.  Zero on-chip transposes.
"""

import math
from contextlib import ExitStack

import numpy as np

import concourse.bass as bass
import concourse.mybir as mybir
import concourse.tile as tile
from concourse.bass_utils import run_bass_kernel_spmd
from concourse.tile_rust import add_dep_helper

P = 128
CH = 512          # free-dim chunk for projection matmuls (one PSUM bank, fp32)
QS = 256          # query rows per attention slot
KS = [16, 12, 8, 4]          # k-tiles per slot (uniform SPMD schedule)
MASK_TILES = 4               # last 4 k-tiles of every slot get masked
BLOCKS = ([7, 4, 3, 0], [6, 5, 2, 1])   # 256-row block ids per core parity


def _chunks(total, size):
    return [(o, min(size, total - o)) for o in range(0, total, size)]


def _fix_matmul_waits(nc):
    """Walrus codegen has a small per-instruction sync-wait slot budget (one
    for a self-loading matmul's LDWEIGHTS half, similar for ACT etc).  Move
    extra waits onto NoOps inserted just before the instruction on the same
    engine -- per-engine program order (and thus semantics) is unchanged."""
    skip = (mybir.InstEventSemaphore, mybir.InstNoOp,
            mybir.InstUnconditionalBranch, mybir.InstCall)
    for func in nc.m.functions:
        for bb in func.blocks:
            il = bb.instructions
            new = []
            changed = False
            for inst in il:
                si = getattr(inst, "sync_info", None)
                if (si and si.on_wait and len(si.on_wait) > 1
                        and not isinstance(inst, skip)):
                    waits = list(si.on_wait)
                    for wi, w in enumerate(waits[:-1]):
                        nop = mybir.InstNoOp(
                            name=f"{inst.name}-wfix{wi}", engine=inst.engine,
                            sync_info=mybir.SyncInfo(on_wait=[w], on_update=[]),
                            text_hint="waitfix")
                        new.append(nop)
                    inst.sync_info = mybir.SyncInfo(
                        on_wait=[waits[-1]], on_update=list(si.on_update or []))
                    changed = True
                new.append(inst)
            if changed:
                bb.instructions = new


def build(N=2048, D=1024, fix_waits=True, **bass_kwargs):
    NT = N // P          # number of 128-row key tiles (16)
    DN = D // P          # contraction tiles / e-tiles (8)
    QTOT = N // 2        # query rows per core (1024)
    NSLOT = QTOT // QS   # attention slots (4)
    SCALE = 1.0 / math.sqrt(N)
    BF = mybir.dt.bfloat16
    F32 = mybir.dt.float32
    AF = mybir.ActivationFunctionType
    OP = mybir.AluOpType

    nc = bass.Bass(**bass_kwargs)
    anchors = []  # first K-proj matmul of each ctx chunk; DMA stage gates
    kends = []    # last K-proj matmul of each ctx chunk

    def _after(dma_bi, anchor_idx, lst=None):
        """Gate a bulk DMA behind an earlier compute anchor so concurrent
        transfers don't fair-share-starve the startup-critical ones."""
        lst = anchors if lst is None else lst
        if lst and anchor_idx < len(lst):
            add_dep_helper(dma_bi.ins, lst[anchor_idx].ins, sync=True,
                           reason="dma staging")
        return dma_bi

    ctx_kvT = nc.declare_dram_parameter("ctx_kvT", [D, N], BF, isOutput=False)
    ctx_qT = nc.declare_dram_parameter("ctx_qT", [D, QTOT], BF, isOutput=False)
    w_qkv = nc.declare_dram_parameter("w_qkv", [D, 3 * D], BF, isOutput=False)
    qpos = nc.declare_dram_parameter("qpos", [P, QTOT], F32, isOutput=False)
    kpos = nc.declare_dram_parameter("kpos", [P, NT], F32, isOutput=False)
    bqT = nc.declare_dram_parameter("bqT", [P, DN], F32, isOutput=False)
    bkT = nc.declare_dram_parameter("bkT", [P, DN], F32, isOutput=False)
    bvb = nc.declare_dram_parameter("bvb", [P, D], F32, isOutput=False)
    onesd = nc.declare_dram_parameter("onesd", [P, 8], BF, isOutput=False)
    out_ext = nc.declare_dram_parameter("out", [QTOT, D], BF, isOutput=True)

    with ExitStack() as ctx:
        tc = ctx.enter_context(tile.TileContext(nc))
        const = ctx.enter_context(tc.tile_pool(name="const", bufs=1))
        persist = ctx.enter_context(tc.tile_pool(name="persist", bufs=1))

        # PE warmup: dummy matmuls on a memset tile bridge the DMA-dead
        # startup window so the HAM clock gate is at 8/8 when real data lands
        # (otherwise the first ~10us of projection run at 1.2 GHz).  Rotating
        # PSUM tiles keep the stream gap-free so the busy-window flips early.
        with tc.tile_pool(name="warm", bufs=1) as warmp, \
             tc.tile_pool(name="warmps", bufs=4, space="PSUM") as warmpp:
            wtile = warmp.tile([P, 3 * P], BF)
            nc.vector.memset(wtile, 0.0)
            for _ in range(44):
                wps = warmpp.tile([P, 3 * P], F32, tag="w", name="wps")
                nc.tensor.matmul(wps, lhsT=wtile[:, 0:P], rhs=wtile, start=True, stop=True)

        bq_sb = const.tile([P, DN], F32)
        nc.sync.dma_start(out=bq_sb, in_=bqT[:, :])
        bk_sb = const.tile([P, DN], F32)
        nc.sync.dma_start(out=bk_sb, in_=bkT[:, :])
        ones_sb = const.tile([P, 8], BF)
        nc.sync.dma_start(out=ones_sb, in_=onesd[:, :])
        kpos_sb = const.tile([P, NT], F32)
        nc.sync.dma_start(out=kpos_sb, in_=kpos[:, :])
        qpos_sb = const.tile([P, QTOT], F32)
        bv_sb = const.tile([P, D], F32)

        # K^T and V both SBUF-resident for the whole kernel (bf16 makes room).
        keT = [persist.tile([P, N], BF, tag=f"ke{e}", name=f"ke{e}") for e in range(DN)]
        v_sb = [persist.tile([P, D], BF, tag=f"v{t}", name=f"v{t}") for t in range(NT)]

        # Q-phase tiles live in outer pools so their DMAs can be issued early,
        # interleaved with the KV-phase transfers on the queue.
        wqp = ctx.enter_context(tc.tile_pool(name="wq", bufs=1))
        ctxq = ctx.enter_context(tc.tile_pool(name="ctxq", bufs=1))
        wq_sb = [wqp.tile([P, D], BF, tag=f"wq{d}", name=f"wq{d}") for d in range(DN)]
        cq_sb = [ctxq.tile([P, QTOT], BF, tag=f"cq{d}", name=f"cq{d}") for d in range(DN)]

        # ---------------- K/V projection (ctx_kvT read once) ----------------
        # Per 512-key chunk: K-proj e-tiles, then V-proj key-tiles.  DMA
        # deadlines under 8-core HBM contention (~170GB/s/core): wk halves by
        # ~19us, wv by ~26us, ctx chunk ci by ~13.6us*ci after chunk-0 start.
        with tc.tile_pool(name="wkv", bufs=1) as wkv, \
             tc.tile_pool(name="ctxp", bufs=1) as ctxp, \
             tc.tile_pool(name="pp", bufs=8, space="PSUM") as pp:
            wk_sb = [wkv.tile([P, D], BF, tag=f"wk{d}", name=f"wk{d}") for d in range(DN)]
            wv_sb = [wkv.tile([P, D], BF, tag=f"wv{d}", name=f"wv{d}") for d in range(DN)]
            # startup-critical DMA order: W_k first halves (sync ring), first
            # ctx chunk (gpsimd ring, in parallel), then the rest chained
            HF = D // 2
            for d in range(DN):
                nc.sync.dma_start(out=wk_sb[d][:, 0:HF], in_=w_qkv[d * P:(d + 1) * P, D:D + HF])
            first_cts = []
            for d in range(DN):
                ct = ctxp.tile([P, CH], BF, tag=f"ct0_{d}", name=f"ct0_{d}")
                stage0_last = nc.gpsimd.dma_start(ct, ctx_kvT[d * P:(d + 1) * P, 0:CH])
                first_cts.append(ct)
            for d in range(DN):
                wk2 = nc.sync.dma_start(out=wk_sb[d][:, HF:D], in_=w_qkv[d * P:(d + 1) * P, D + HF:2 * D])
                add_dep_helper(wk2.ins, stage0_last.ins, sync=True, reason="dma staging")
            # W_v in column halves: the V-pass consumes e-chunk 0 first, so
            # the second half gets ~3.5us more slack against the DMA stream
            for d in range(DN):
                wva = nc.sync.dma_start(out=wv_sb[d][:, 0:HF], in_=w_qkv[d * P:(d + 1) * P, 2 * D:2 * D + HF])
                add_dep_helper(wva.ins, wk2.ins, sync=True, reason="dma staging")
            for d in range(DN):
                wvb = nc.sync.dma_start(out=wv_sb[d][:, HF:D], in_=w_qkv[d * P:(d + 1) * P, 2 * D + HF:3 * D])
                add_dep_helper(wvb.ins, wva.ins, sync=True, reason="dma staging")
            bvd = nc.sync.dma_start(out=bv_sb, in_=bvb[:, :])
            add_dep_helper(bvd.ins, wvb.ins, sync=True, reason="dma staging")

            chunks = _chunks(N, CH)
            for ci, (coff, csz) in enumerate(chunks):
                if ci == 0:
                    cts = first_cts
                else:
                    cts = []
                    for d in range(DN):
                        ct = ctxp.tile([P, CH], BF, tag=f"ct{ci}_{d}", name=f"ct{ci}_{d}")
                        _after(nc.sync.dma_start(out=ct[:, :csz],
                                                 in_=ctx_kvT[d * P:(d + 1) * P, coff:coff + csz]), ci - 1)
                        cts.append(ct)
                    if ci == 1:        # Q-phase weights: land by ~mid-KV
                        for d in range(DN):
                            _after(nc.sync.dma_start(out=wq_sb[d], in_=w_qkv[d * P:(d + 1) * P, 0:D]), 0)
                    if ci == 2:        # Q-phase context + positions
                        for d in range(DN):
                            _after(nc.sync.dma_start(out=cq_sb[d], in_=ctx_qT[d * P:(d + 1) * P, :]), 1)
                        _after(nc.sync.dma_start(out=qpos_sb, in_=qpos[:, :]), 1)
                for e in range(DN):
                    psk = pp.tile([P, CH], F32, tag="pp8", name="psk")
                    for d in range(DN):
                        mm = nc.tensor.matmul(psk[:, :csz], lhsT=wk_sb[d][:, e * P:(e + 1) * P],
                                              rhs=cts[d][:, :csz], start=(d == 0), stop=(d == DN - 1))
                        if e == 0 and d == 0:
                            anchors.append(mm)
                        if e == DN - 1 and d == DN - 1:
                            kends.append(mm)
                    nc.scalar.activation(keT[e][:, coff:coff + csz], psk[:, :csz],
                                         AF.Identity, bias=bk_sb[:, e:e + 1], scale=1.0)
                for eoff, esz in _chunks(D, CH):
                    for nt_loc in range(csz // P):
                        n_t = coff // P + nt_loc
                        psv = pp.tile([P, CH], F32, tag="pp8", name="psv")
                        for d in range(DN):
                            nc.tensor.matmul(psv[:, :esz],
                                             lhsT=cts[d][:, nt_loc * P:(nt_loc + 1) * P],
                                             rhs=wv_sb[d][:, eoff:eoff + esz], start=(d == 0), stop=(d == DN - 1))
                        nc.vector.tensor_tensor(v_sb[n_t][:, eoff:eoff + esz], psv[:, :esz],
                                                bv_sb[:, eoff:eoff + esz], OP.add)

        # ---------------- Q projection + attention slots ----------------
        with tc.tile_pool(name="qtb", bufs=1) as qtb, \
             tc.tile_pool(name="att_e", bufs=2) as epool, \
             tc.tile_pool(name="att_m", bufs=3) as mpool, \
             tc.tile_pool(name="att_o", bufs=3) as opool, \
             tc.tile_pool(name="ps_s", bufs=2, space="PSUM") as ps_s, \
             tc.tile_pool(name="ps_pv", bufs=4, space="PSUM") as ps_pv, \
             tc.tile_pool(name="ps_den", bufs=2, space="PSUM") as ps_den:
            # Q^T for all 1024 local query rows, in [e, q] layout
            qT_sb = [qtb.tile([P, QTOT], BF, tag=f"qtb{e}", name=f"qtb{e}") for e in range(DN)]
            for qoff, qsz in _chunks(QTOT, CH):
                for e in range(DN):
                    psq = ps_s.tile([P, CH], F32, tag="s", name="psq")
                    for d in range(DN):
                        nc.tensor.matmul(psq[:, :qsz], lhsT=wq_sb[d][:, e * P:(e + 1) * P],
                                         rhs=cq_sb[d][:, qoff:qoff + qsz], start=(d == 0), stop=(d == DN - 1))
                    nc.scalar.activation(qT_sb[e][:, qoff:qoff + qsz], psq[:, :qsz],
                                         AF.Identity, bias=bq_sb[:, e:e + 1], scale=1.0)

            for s in range(NSLOT):
                KT = KS[s]
                qr0 = s * QS
                e_sb = [epool.tile([P, QS], BF, tag=f"e{k}", name=f"e{k}") for k in range(KT)]
                # scores + exp (+ mask on the last MASK_TILES k-tiles)
                for k in range(KT):
                    pss = ps_s.tile([P, QS], F32, tag="s", name="pss")
                    for d in range(DN):
                        nc.tensor.matmul(pss, lhsT=keT[d][:, k * P:(k + 1) * P],
                                         rhs=qT_sb[d][:, qr0:qr0 + QS], start=(d == 0), stop=(d == DN - 1))
                    nc.scalar.activation(e_sb[k], pss, AF.Exp, scale=SCALE)
                    if k >= KT - MASK_TILES:
                        m = mpool.tile([P, QS], BF, tag="m", name="m")
                        nc.vector.tensor_scalar(m, qpos_sb[:, qr0:qr0 + QS],
                                                kpos_sb[:, k:k + 1], None, OP.is_ge)
                        nc.vector.tensor_tensor(e_sb[k], e_sb[k], m, OP.mult)
                # PV per 128-row q-tile (V is SBUF-resident: no DMA here).
                # Denominator first, then e-chunk 0 (scaled on ScalarE and
                # DMA'd while e-chunk 1 is still in the matmul pipe), then
                # e-chunk 1 (VectorE) -- keeps the end-of-kernel chain short.
                for qt in range(QS // P):
                    psd = ps_den.tile([P, 8], F32, tag="den", name="psd")
                    for k in range(KT):
                        nc.tensor.matmul(psd, lhsT=e_sb[k][:, qt * P:(qt + 1) * P], rhs=ones_sb,
                                         start=(k == 0), stop=(k == KT - 1))
                    rec = mpool.tile([P, 1], F32, tag="rec", name="rec")
                    nc.vector.reciprocal(rec, psd[:, 0:1])
                    for ei, (eoff, esz) in enumerate(_chunks(D, CH)):
                        pso = ps_pv.tile([P, CH], F32, tag="pv", name="pso")
                        for k in range(KT):
                            nc.tensor.matmul(pso[:, :esz], lhsT=e_sb[k][:, qt * P:(qt + 1) * P],
                                             rhs=v_sb[k][:, eoff:eoff + esz],
                                             start=(k == 0), stop=(k == KT - 1))
                        ot = opool.tile([P, CH], BF, tag="o", name="ot")
                        orow = out_ext[qr0 + qt * P:qr0 + (qt + 1) * P, :]
                        if ei == 0:
                            nc.scalar.activation(ot[:, :esz], pso[:, :esz], AF.Identity, scale=rec)
                            nc.gpsimd.dma_start(out=orow[:, eoff:eoff + esz], in_=ot[:, :esz])
                        else:
                            nc.vector.tensor_scalar_mul(ot[:, :esz], pso[:, :esz], rec)
                            nc.sync.dma_start(out=orow[:, eoff:eoff + esz], in_=ot[:, :esz])
    if fix_waits:
        _fix_matmul_waits(nc)
    return nc


def make_in_maps(context, W_qkv, b_qkv, n_cores=8):
    import ml_dtypes
    bf16 = ml_dtypes.bfloat16
    context = np.ascontiguousarray(np.asarray(context, np.float32))
    W_qkv = np.ascontiguousarray(np.asarray(W_qkv, np.float32))
    b_qkv = np.ascontiguousarray(np.asarray(b_qkv, np.float32))
    B, N, D = context.shape
    NT = N // P
    DN = D // P
    kpos = (np.arange(NT)[None, :] * P + np.arange(P)[:, None]).astype(np.float32)
    kpos = np.ascontiguousarray(kpos)
    bq = np.ascontiguousarray(b_qkv[0:D].reshape(DN, P).T)
    bk = np.ascontiguousarray(b_qkv[D:2 * D].reshape(DN, P).T)
    bv = np.ascontiguousarray(np.broadcast_to(b_qkv[2 * D:3 * D], (P, D)))
    w_bf = np.ascontiguousarray(W_qkv.astype(bf16))
    in_maps = []
    for c in range(n_cores):
        b, j = divmod(c, 2)
        blocks = BLOCKS[j]
        ctx_b = context[b]
        ctx_kvT = np.ascontiguousarray(ctx_b.T.astype(bf16))
        rows = np.concatenate([np.arange(i * QS, (i + 1) * QS) for i in blocks])
        ctx_qT = np.ascontiguousarray(ctx_b[rows].T.astype(bf16))
        qpos_b = np.ascontiguousarray(
            np.broadcast_to(rows.astype(np.float32), (P, rows.size)))
        in_maps.append({
            "ctx_kvT": ctx_kvT, "ctx_qT": ctx_qT, "w_qkv": w_bf,
            "qpos": qpos_b, "kpos": kpos, "bqT": bq, "bkT": bk, "bvb": bv,
            "onesd": np.ones((P, 8), bf16),
        })
    return in_maps


def assemble(results, B, N, D):
    out = np.zeros((B, N, D), np.float32)
    for c, res in enumerate(results):
        b, j = divmod(c, 2)
        o = np.asarray(res["out"], np.float32)
        for s, i in enumerate(BLOCKS[j]):
            out[b, i * QS:(i + 1) * QS] = o[s * QS:(s + 1) * QS]
    return out


def run(inputs, trace=False, **spmd_kwargs):
    context = np.asarray(inputs["context"])
    B, N, D = context.shape
    nc = build(N, D)
    in_maps = make_in_maps(context, inputs["W_qkv"], inputs["b_qkv"], n_cores=8)
    res = run_bass_kernel_spmd(nc, in_maps, core_ids=list(range(8)), trace=trace, **spmd_kwargs)
    out = assemble(res.results, B, N, D)
    return out, res


def kernel(context, W_qkv, b_qkv):
    out, _ = run({"context": context, "W_qkv": W_qkv, "b_qkv": b_qkv})
    return out



# revision 3
# speedup vs baseline: 1.0741x; 1.0741x over previous
"""Causal self-attention (QKV projection + softmax(QK^T/sqrt(N)) @ V) on 8 TRN2
NeuronCores — fp8 DoubleRow edition.

Sharding as the bf16 ancestor: core c = 2*b + j handles batch element b and
half its query rows as four 256-row blocks, interleaved for causal load
balance: j=0 -> [7,4,3,0], j=1 -> [6,5,2,1]; SPMD slot k-tile counts
KS=[16,12,8,4] with position-mask cleanup on the last 4 k-tiles per slot.

fp8 (e4m3) DoubleRow doubles PE throughput (2 contraction elements per cell
per cycle) on the projection matmuls.  Numerics (validated offline, rel-err
7.1e-3 vs 2e-2 budget): quantization noise in q/k/v is harmless for query
rows >= 512 (softmax averaging washes it out: error ~ eps*sqrt(sum w^2)), but
early rows expose single V rows and logit noise directly.  So keys 0-511 and
query rows 0-511 (slot 3 + K/V chunk 0) stay on the bf16 path; keys/queries
512+ use fp8 DoubleRow projections.  Projection OUTPUTS stay bf16 (scores and
PV are bf16 here).

DoubleRow operand layout: [128 part, 2, F] — dim 1 selects the contraction
pair member (d-tile 2*dp / 2*dp+1), host pre-pairs both W and ctx.
"""

import math
from contextlib import ExitStack

import numpy as np

import concourse.bass as bass
import concourse.mybir as mybir
import concourse.tile as tile
from concourse.bass_utils import run_bass_kernel_spmd
from concourse.tile_rust import add_dep_helper

P = 128
CH = 512          # free-dim chunk for projection matmuls (one PSUM bank, fp32)
QS = 256          # query rows per attention slot
KS = [16, 12, 8, 4]          # k-tiles per slot (uniform SPMD schedule)
MASK_TILES = 4               # last 4 k-tiles of every slot get masked
BLOCKS = ([7, 4, 3, 0], [6, 5, 2, 1])   # 256-row block ids per core parity
QF8 = 768         # query rows on the fp8 path (slots 0-2)


def _chunks(total, size):
    return [(o, min(size, total - o)) for o in range(0, total, size)]


def _fix_matmul_waits(nc):
    """Walrus codegen has a small per-instruction sync-wait slot budget (one
    for a self-loading matmul's LDWEIGHTS half, similar for ACT etc).  Move
    extra waits onto NoOps inserted just before the instruction on the same
    engine -- per-engine program order (and thus semantics) is unchanged."""
    skip = (mybir.InstEventSemaphore, mybir.InstNoOp,
            mybir.InstUnconditionalBranch, mybir.InstCall)
    for func in nc.m.functions:
        for bb in func.blocks:
            il = bb.instructions
            new = []
            changed = False
            for inst in il:
                si = getattr(inst, "sync_info", None)
                if (si and si.on_wait and len(si.on_wait) > 1
                        and not isinstance(inst, skip)):
                    waits = list(si.on_wait)
                    for wi, w in enumerate(waits[:-1]):
                        nop = mybir.InstNoOp(
                            name=f"{inst.name}-wfix{wi}", engine=inst.engine,
                            sync_info=mybir.SyncInfo(on_wait=[w], on_update=[]),
                            text_hint="waitfix")
                        new.append(nop)
                    inst.sync_info = mybir.SyncInfo(
                        on_wait=[waits[-1]], on_update=list(si.on_update or []))
                    changed = True
                new.append(inst)
            if changed:
                bb.instructions = new


def build(N=2048, D=1024, fix_waits=True, **bass_kwargs):
    NT = N // P          # number of 128-row key tiles (16)
    DN = D // P          # contraction tiles / e-tiles (8)
    DP = DN // 2         # contraction pair-tiles for DoubleRow (4)
    QTOT = N // 2        # query rows per core (1024)
    NSLOT = QTOT // QS   # attention slots (4)
    SCALE = 1.0 / math.sqrt(N)
    BF = mybir.dt.bfloat16
    F8 = mybir.dt.float8e4
    F32 = mybir.dt.float32
    AF = mybir.ActivationFunctionType
    OP = mybir.AluOpType
    DR = mybir.MatmulPerfMode.DoubleRow

    nc = bass.Bass(**bass_kwargs)
    anchors = []  # first K-proj matmul of each ctx chunk; DMA stage gates

    def _dep(dma_bi, on):
        add_dep_helper(dma_bi.ins, on.ins, sync=True, reason="dma staging")
        return dma_bi

    def _after(dma_bi, anchor_idx):
        if anchor_idx < len(anchors):
            _dep(dma_bi, anchors[anchor_idx])
        return dma_bi

    # bf16 operands (keys 0-511 / slot-3 queries)
    ctx0 = nc.declare_dram_parameter("ctx0", [D, QS], BF, isOutput=False)
    cq0 = nc.declare_dram_parameter("cq0", [D, QS], BF, isOutput=False)
    wk0 = nc.declare_dram_parameter("wk0", [D, D], BF, isOutput=False)
    wv0 = nc.declare_dram_parameter("wv0", [D, D], BF, isOutput=False)
    wq0 = nc.declare_dram_parameter("wq0", [D, D], BF, isOutput=False)
    # fp8 DoubleRow operands, contraction-paired [.., dp*128+p, i, :].
    # Weights are split into column-half blocks and ctx into key-chunk blocks
    # so every DMA tile is a contiguous DRAM region (1KB+ rows -> full DMA
    # descriptor efficiency; a flat pair layout would leave 512B strided
    # segments and halve effective HBM bandwidth).
    ctx8 = nc.declare_dram_parameter("ctx8", [NT * P // CH - 1, DP * P, 2, CH], F8, isOutput=False)
    ctx8b = nc.declare_dram_parameter("ctx8b", [DP * P, 2, QS], F8, isOutput=False)
    cq8 = nc.declare_dram_parameter("cq8", [DP * P, 2, QF8], F8, isOutput=False)
    wk8 = nc.declare_dram_parameter("wk8", [4, DP * P, 2, QS], F8, isOutput=False)
    wv8 = nc.declare_dram_parameter("wv8", [2, DP * P, 2, CH], F8, isOutput=False)
    wq8 = nc.declare_dram_parameter("wq8", [2, DP * P, 2, CH], F8, isOutput=False)

    qpos = nc.declare_dram_parameter("qpos", [P, QTOT], F32, isOutput=False)
    kpos = nc.declare_dram_parameter("kpos", [P, NT], F32, isOutput=False)
    bqT = nc.declare_dram_parameter("bqT", [P, DN], F32, isOutput=False)
    bkT = nc.declare_dram_parameter("bkT", [P, DN], F32, isOutput=False)
    bvb = nc.declare_dram_parameter("bvb", [P, D], F32, isOutput=False)
    out_ext = nc.declare_dram_parameter("out", [QTOT, D], BF, isOutput=True)

    with ExitStack() as ctx:
        tc = ctx.enter_context(tile.TileContext(nc))
        const = ctx.enter_context(tc.tile_pool(name="const", bufs=1))
        persist = ctx.enter_context(tc.tile_pool(name="persist", bufs=1))

        # PE warmup: dummy matmuls on a memset tile bridge the DMA-dead
        # startup window so the HAM clock gate is at 8/8 when real data lands.
        with tc.tile_pool(name="warm", bufs=1) as warmp, \
             tc.tile_pool(name="warmps", bufs=4, space="PSUM") as warmpp:
            wtile = warmp.tile([P, 3 * P], BF)
            nc.vector.memset(wtile, 0.0)
            for _ in range(30):
                wps = warmpp.tile([P, 3 * P], F32, tag="w", name="wps")
                nc.tensor.matmul(wps, lhsT=wtile[:, 0:P], rhs=wtile, start=True, stop=True)

        bq_sb = const.tile([P, DN], F32)
        nc.sync.dma_start(out=bq_sb, in_=bqT[:, :])
        bk_sb = const.tile([P, DN], F32)
        nc.sync.dma_start(out=bk_sb, in_=bkT[:, :])
        kpos_sb = const.tile([P, NT], F32)
        nc.sync.dma_start(out=kpos_sb, in_=kpos[:, :])
        qpos_sb = const.tile([P, QTOT], F32)
        bv_sb = const.tile([P, D], F32)

        # K^T and V both SBUF-resident for the whole kernel.  V tiles carry 16
        # extra ones-columns: the PV matmul then accumulates the softmax
        # denominator for free in its last output chunk (no separate
        # denominator matmuls).  v8 = fp8 contraction-paired V (all 16 k-tiles,
        # for the fp8 DoubleRow PV of slots 0-2); v_sb = bf16 V (k-tiles 0-3,
        # for slot 3's bf16 PV).
        DV = D + 16
        # K^T fp8 e-paired (all 16 k-tiles, for fp8 scores of slots 0-2) +
        # bf16 K^T for k-tiles 0-3 only (slot 3's bf16 scores).
        keT8 = [persist.tile([P, 2, N], F8, tag=f"ke8{ep}", name=f"ke8{ep}") for ep in range(DN // 2)]
        keT = [persist.tile([P, CH], BF, tag=f"ke{e}", name=f"ke{e}") for e in range(DN)]
        v_sb = [persist.tile([P, DV], BF, tag=f"v{t}", name=f"v{t}") for t in range(4)]
        v8_sb = [persist.tile([P, 2, DV], F8, tag=f"v8{tp}", name=f"v8{tp}") for tp in range(NT // 2)]
        for t in range(4):
            nc.vector.memset(v_sb[t][:, D:DV], 1.0)
        for tp in range(NT // 2):
            nc.vector.memset(v8_sb[tp][:, 0, D:DV], 1.0)
            nc.vector.memset(v8_sb[tp][:, 1, D:DV], 1.0)

        # Q-phase tiles live in outer pools so their DMAs can be issued early.
        wqp = ctx.enter_context(tc.tile_pool(name="wq", bufs=1))
        ctxq = ctx.enter_context(tc.tile_pool(name="ctxq", bufs=1))
        wq8_sb = [[wqp.tile([P, 2, CH], F8, tag=f"wq8{h}{dp}", name=f"wq8{h}{dp}")
                   for dp in range(DP)] for h in range(2)]
        wq0_sb = [wqp.tile([P, D], BF, tag=f"wq0{d}", name=f"wq0{d}") for d in range(DN)]
        cq8_sb = [ctxq.tile([P, 2, QF8], F8, tag=f"cq8{dp}", name=f"cq8{dp}") for dp in range(DP)]
        cq0_sb = [ctxq.tile([P, QS], BF, tag=f"cq0{d}", name=f"cq0{d}") for d in range(DN)]

        # ---------------- K/V projection ----------------
        # ALL K chunks first (fp8 keys 512.. then bf16 keys 0-511), THEN all V
        # chunks: every V operand deadline moves ~40us later, so the startup
        # DMA stream only has to race the K weights.  wk8 ships in quarter
        # column-blocks so the first matmul waits for just 0.25MB + ctx.
        with tc.tile_pool(name="wkv", bufs=1) as wkv, \
             tc.tile_pool(name="ctxp", bufs=1) as ctxp, \
             tc.tile_pool(name="pp", bufs=6, space="PSUM") as pp:
            wk8_sb = [[wkv.tile([P, 2, QS], F8, tag=f"wk8{qq}{dp}", name=f"wk8{qq}{dp}")
                       for dp in range(DP)] for qq in range(4)]
            wv8_sb = [[wkv.tile([P, 2, CH], F8, tag=f"wv8{h}{dp}", name=f"wv8{h}{dp}")
                       for dp in range(DP)] for h in range(2)]
            wk0_sb = [wkv.tile([P, D], BF, tag=f"wk0{d}", name=f"wk0{d}") for d in range(DN)]
            wv0_sb = [wkv.tile([P, D], BF, tag=f"wv0{d}", name=f"wv0{d}") for d in range(DN)]

            HF = D // 2
            # startup-critical: wk8 quarter 0 (sync ring) + ctx8 chunk 1
            # (gpsimd ring) in parallel; later batches issue in parallel
            # within a batch, chained batch-to-batch + compute-anchor gates.
            # Within a ring, program order IS execution order — no completion
            # deps needed (they only add a latency bubble per batch).  Anchor
            # gates alone hold lower-priority transfers back.
            cts8 = {}
            for dp in range(DP):
                ct = ctxp.tile([P, 2, CH], F8, tag=f"c8_1_{dp}", name=f"c8_1_{dp}")
                nc.gpsimd.dma_start(out=ct, in_=ctx8[0, dp * P:(dp + 1) * P, :, :])
                cts8[(1, dp)] = ct
            for qq in range(4):
                for dp in range(DP):
                    nc.sync.dma_start(out=wk8_sb[qq][dp], in_=wk8[qq, dp * P:(dp + 1) * P, :, :])
            nc.sync.dma_start(out=bv_sb, in_=bvb[:, :])

            def k_chunk_fp8(ci, merge_b=False):
                # merge_b: fold keys 256-511 in as a second moving group that
                # reuses each stationary load (alone they'd be LDW-bound).
                coff = ci * CH
                cts = [cts8[(ci, dp)] for dp in range(DP)]
                cb8 = [cts8[("b", dp)] for dp in range(DP)] if merge_b else None
                for e in range(DN):
                    psk = pp.tile([P, CH], F32, tag="pp8", name="psk")
                    psB = pp.tile([P, QS], F32, tag="ppB", name="psB", bufs=2) if merge_b else None
                    for dp in range(DP):
                        lhs = wk8_sb[e // 2][dp][:, :, (e % 2) * P:(e % 2 + 1) * P]
                        mm = nc.tensor.matmul(psk, lhsT=lhs, rhs=cts[dp],
                                              start=(dp == 0), stop=(dp == DP - 1),
                                              perf_mode=DR)
                        if e == 0 and dp == 0:
                            anchors.append(mm)
                        if merge_b:
                            nc.tensor.matmul(psB, lhsT=lhs, rhs=cb8[dp],
                                             start=(dp == 0), stop=(dp == DP - 1),
                                             perf_mode=DR)
                    nc.scalar.activation(keT8[e // 2][:, e % 2, coff:coff + CH], psk,
                                         AF.Identity, bias=bk_sb[:, e:e + 1], scale=1.0)
                    if merge_b:
                        nc.scalar.activation(keT8[e // 2][:, e % 2, QS:CH], psB,
                                             AF.Identity, bias=bk_sb[:, e:e + 1], scale=1.0)
                        nc.vector.tensor_scalar(keT[e][:, QS:CH], psB,
                                                bk_sb[:, e:e + 1], None, OP.add)

            def k_chunk_bf16():
                cts = [cts8[(0, d)] for d in range(DN)]
                for e in range(DN):
                    psk = pp.tile([P, QS], F32, tag="ppB", name="psk0", bufs=2)
                    for d in range(DN):
                        mm = nc.tensor.matmul(psk, lhsT=wk0_sb[d][:, e * P:(e + 1) * P],
                                              rhs=cts[d], start=(d == 0), stop=(d == DN - 1))
                        if e == 0 and d == 0:
                            anchors.append(mm)
                    nc.scalar.activation(keT[e][:, 0:QS], psk,
                                         AF.Identity, bias=bk_sb[:, e:e + 1], scale=1.0)
                    nc.vector.tensor_scalar(keT8[e // 2][:, e % 2, 0:QS], psk,
                                            bk_sb[:, e:e + 1], None, OP.add)

            def v_chunk_fp8(ci):
                coff = ci * CH
                cts = [cts8[(ci, dp)] for dp in range(DP)]
                for eoff, esz in _chunks(D, CH):
                    for ntl in range(CH // P):
                        n_t = coff // P + ntl
                        psv = pp.tile([P, CH], F32, tag="pp8", name="psv")
                        for dp in range(DP):
                            nc.tensor.matmul(psv[:, :esz], lhsT=cts[dp][:, :, ntl * P:(ntl + 1) * P],
                                             rhs=wv8_sb[eoff // CH][dp],
                                             start=(dp == 0), stop=(dp == DP - 1), perf_mode=DR)
                        nc.vector.tensor_tensor(v8_sb[n_t // 2][:, n_t % 2, eoff:eoff + esz],
                                                psv[:, :esz], bv_sb[:, eoff:eoff + esz], OP.add)

            def v_chunk_bf16():
                # V keys 0-255 bf16, keys 256-511 fp8 DoubleRow (v-noise only
                # reaches rows >= 256, where softmax averaging absorbs it).
                # Both halves dual-write the bf16 copy (slot 3's PV) and the
                # fp8 paired copy (slots 0-2).
                cts = [cts8[(0, d)] for d in range(DN)]
                cb8 = [cts8[("b", dp)] for dp in range(DP)]
                for eoff, esz in _chunks(D, CH):
                    for ntl in range(QS // P):
                        psv = pp.tile([P, CH], F32, tag="pp8", name="psv")
                        for d in range(DN):
                            nc.tensor.matmul(psv[:, :esz], lhsT=cts[d][:, ntl * P:(ntl + 1) * P],
                                             rhs=wv0_sb[d][:, eoff:eoff + esz],
                                             start=(d == 0), stop=(d == DN - 1))
                        nc.vector.tensor_tensor(v_sb[ntl][:, eoff:eoff + esz], psv[:, :esz],
                                                bv_sb[:, eoff:eoff + esz], OP.add)
                        # duplicate fp8 copy from the finished bf16 tile on the
                        # (idle) scalar engine — a second vector read of psum
                        # would make this phase DVE-bound
                        nc.scalar.activation(v8_sb[0][:, ntl, eoff:eoff + esz],
                                             v_sb[ntl][:, eoff:eoff + esz], AF.Identity, scale=1.0)
                    for ntl in range(QS // P):
                        psv = pp.tile([P, CH], F32, tag="pp8", name="psv")
                        for dp in range(DP):
                            nc.tensor.matmul(psv[:, :esz], lhsT=cb8[dp][:, :, ntl * P:(ntl + 1) * P],
                                             rhs=wv8_sb[eoff // CH][dp],
                                             start=(dp == 0), stop=(dp == DP - 1), perf_mode=DR)
                        nc.vector.tensor_tensor(v_sb[2 + ntl][:, eoff:eoff + esz], psv[:, :esz],
                                                bv_sb[:, eoff:eoff + esz], OP.add)
                        nc.scalar.activation(v8_sb[1][:, ntl, eoff:eoff + esz],
                                             v_sb[2 + ntl][:, eoff:eoff + esz], AF.Identity, scale=1.0)

            for ci in (1, 2, 3):
                if ci > 1:      # ctx8 chunk ci: plain ring order behind c1
                    for dp in range(DP):
                        ct = ctxp.tile([P, 2, CH], F8, tag=f"c8_{ci}_{dp}", name=f"c8_{ci}_{dp}")
                        nc.gpsimd.dma_start(out=ct, in_=ctx8[ci - 1, dp * P:(dp + 1) * P, :, :])
                        cts8[(ci, dp)] = ct
                if ci == 2:     # keys 256-511 (K3 B-group + V0), bf16 K operands
                    for dp in range(DP):
                        ct = ctxp.tile([P, 2, QS], F8, tag=f"c8b_{dp}", name=f"c8b_{dp}")
                        _after(nc.sync.dma_start(out=ct, in_=ctx8b[dp * P:(dp + 1) * P, :, :]), 0)
                        cts8[("b", dp)] = ct
                    for d in range(DN):
                        _after(nc.sync.dma_start(
                            out=wk0_sb[d][:, 0:HF], in_=wk0[d * P:(d + 1) * P, 0:HF]), 1)
                    for d in range(DN):
                        ct = ctxp.tile([P, QS], BF, tag=f"c0_{d}", name=f"c0_{d}")
                        _after(nc.gpsimd.dma_start(out=ct, in_=ctx0[d * P:(d + 1) * P, :]), 1)
                        cts8[(0, d)] = ct
                    for d in range(DN):
                        _after(nc.sync.dma_start(
                            out=wk0_sb[d][:, HF:D], in_=wk0[d * P:(d + 1) * P, HF:D]), 2)
                k_chunk_fp8(ci, merge_b=(ci == 3))
            k_chunk_bf16()
            # V-phase + Q-phase operands, all gated well behind their use
            for h in range(2):
                for d in range(DN):
                    _after(nc.sync.dma_start(
                        out=wv0_sb[d][:, h * HF:(h + 1) * HF],
                        in_=wv0[d * P:(d + 1) * P, h * HF:(h + 1) * HF]), 2)
            for h in range(2):
                for dp in range(DP):
                    _after(nc.sync.dma_start(
                        out=wv8_sb[h][dp], in_=wv8[h, dp * P:(dp + 1) * P, :, :]), 2)
            v_chunk_bf16()
            for h in range(2):
                for dp in range(DP):
                    _after(nc.sync.dma_start(
                        out=wq8_sb[h][dp], in_=wq8[h, dp * P:(dp + 1) * P, :, :]), 3)
            for dp in range(DP):
                _after(nc.sync.dma_start(out=cq8_sb[dp], in_=cq8[dp * P:(dp + 1) * P, :, :]), 3)
            for ci in (1, 2, 3):
                v_chunk_fp8(ci)
            for d in range(DN):
                _after(nc.sync.dma_start(out=wq0_sb[d], in_=wq0[d * P:(d + 1) * P, :]), 3)
            for d in range(DN):
                _after(nc.sync.dma_start(out=cq0_sb[d], in_=cq0[d * P:(d + 1) * P, :]), 3)
            _after(nc.sync.dma_start(out=qpos_sb, in_=qpos[:, :]), 3)

        # ---------------- Q projection + attention slots ----------------
        with tc.tile_pool(name="qtb", bufs=1) as qtb, \
             tc.tile_pool(name="att_e", bufs=2) as epool, \
             tc.tile_pool(name="att_m", bufs=3) as mpool, \
             tc.tile_pool(name="att_o", bufs=3) as opool, \
             tc.tile_pool(name="ps_s", bufs=2, space="PSUM") as ps_s, \
             tc.tile_pool(name="ps_pv", bufs=2, space="PSUM") as ps_pv:
            # Q^T: fp8 e-paired for rows 0..767 (slots 0-2), bf16 for slot 3.
            qT8 = [qtb.tile([P, 2, QF8], F8, tag=f"qt8{ep}", name=f"qt8{ep}") for ep in range(DN // 2)]
            qT3 = [qtb.tile([P, QS], BF, tag=f"qt3{e}", name=f"qt3{e}") for e in range(DN)]
            # fp8 rows (slots 0-2): both query chunks (cols 0:512 and 512:768)
            # share each stationary load — the 256-col chunk alone would be
            # LDWEIGHTS-bound.
            for e in range(DN):
                psqA = ps_s.tile([P, CH], F32, tag="sA", name="psqA")
                psqB = ps_s.tile([P, QS], F32, tag="sB", name="psqB")
                for dp in range(DP):
                    lhs = wq8_sb[e // 4][dp][:, :, (e % 4) * P:(e % 4 + 1) * P]
                    nc.tensor.matmul(psqA, lhsT=lhs, rhs=cq8_sb[dp][:, :, 0:CH],
                                     start=(dp == 0), stop=(dp == DP - 1), perf_mode=DR)
                    nc.tensor.matmul(psqB, lhsT=lhs, rhs=cq8_sb[dp][:, :, CH:QF8],
                                     start=(dp == 0), stop=(dp == DP - 1), perf_mode=DR)
                nc.scalar.activation(qT8[e // 2][:, e % 2, 0:CH], psqA,
                                     AF.Identity, bias=bq_sb[:, e:e + 1], scale=1.0)
                nc.scalar.activation(qT8[e // 2][:, e % 2, CH:QF8], psqB,
                                     AF.Identity, bias=bq_sb[:, e:e + 1], scale=1.0)
            for e in range(DN):                       # bf16 rows (slot 3)
                psq = ps_s.tile([P, CH], F32, tag="sA", name="psq")
                for d in range(DN):
                    nc.tensor.matmul(psq[:, :QS], lhsT=wq0_sb[d][:, e * P:(e + 1) * P],
                                     rhs=cq0_sb[d], start=(d == 0), stop=(d == DN - 1))
                nc.scalar.activation(qT3[e], psq[:, :QS],
                                     AF.Identity, bias=bq_sb[:, e:e + 1], scale=1.0)

            # ---- fp8 scores for slots 0-2, k-merged ----
            # Slots 0-2 occupy contiguous query columns 0..767, so one pass
            # per k-tile serves every slot that needs it (group A = cols
            # 0..511 -> slots 0+1, group B = cols 512..767 -> slot 2); the
            # DoubleRow stationary (256-col LDWEIGHTS) amortizes across all
            # live query columns instead of being reloaded per slot.
            e8 = [epool.tile([P, 2, QF8], F8, tag=f"e8_{kp}", name=f"e8_{kp}")
                  for kp in range(KS[0] // 2)]

            def mask_e8(k, qc0):
                m = mpool.tile([P, QS], BF, tag="m", name="m")
                nc.vector.tensor_scalar(m, qpos_sb[:, qc0:qc0 + QS],
                                        kpos_sb[:, k:k + 1], None, OP.is_ge)
                nc.vector.tensor_tensor(e8[k // 2][:, k % 2, qc0:qc0 + QS],
                                        e8[k // 2][:, k % 2, qc0:qc0 + QS], m, OP.mult)

            for k in range(KS[0]):
                qnA = CH if k < KS[1] else QS         # slots 0+1, or 0 only
                pssA = ps_s.tile([P, CH], F32, tag="sA", name="pssA")
                pssB = ps_s.tile([P, QS], F32, tag="sB", name="pssB") if k < KS[2] else None
                for ep in range(DN // 2):
                    lhs = keT8[ep][:, :, k * P:(k + 1) * P]
                    nc.tensor.matmul(pssA[:, :qnA], lhsT=lhs, rhs=qT8[ep][:, :, 0:qnA],
                                     start=(ep == 0), stop=(ep == DN // 2 - 1), perf_mode=DR)
                    if pssB is not None:
                        nc.tensor.matmul(pssB, lhsT=lhs, rhs=qT8[ep][:, :, CH:QF8],
                                         start=(ep == 0), stop=(ep == DN // 2 - 1), perf_mode=DR)
                nc.scalar.activation(e8[k // 2][:, k % 2, 0:qnA], pssA[:, :qnA], AF.Exp, scale=SCALE)
                if pssB is not None:
                    nc.scalar.activation(e8[k // 2][:, k % 2, CH:QF8], pssB, AF.Exp, scale=SCALE)
                if k >= KS[2]:                        # mask tail k-tiles per slot
                    s_of_k = 0 if k >= KS[1] else 1
                    mask_e8(k, s_of_k * QS)
                elif k >= KS[2] - MASK_TILES:
                    mask_e8(k, 2 * QS)

            # PV output chunks: (512, 384, 144) — the 144-chunk covers out
            # cols 896..1023 plus the 16 ones-columns, whose accumulated sums
            # ARE the softmax denominator (psum cols 128..143).
            PVCH = [(0, CH), (CH, 384), (896, 144)]

            def pv_store(pso, qr0, qt, last=False):
                # On the final q-tile, send the big chunk down the sync ring
                # (the gpsimd ring then drains earlier, off the critical tail).
                big_dma, small_dma = (nc.sync, nc.gpsimd) if last else (nc.gpsimd, nc.sync)
                rec = mpool.tile([P, 1], F32, tag="rec", name="rec")
                nc.vector.reciprocal(rec, pso[2][:, P:P + 1])
                orow = out_ext[qr0 + qt * P:qr0 + (qt + 1) * P, :]
                ot = opool.tile([P, CH], BF, tag="o", name="ot")
                nc.scalar.activation(ot, pso[0], AF.Identity, scale=rec)
                big_dma.dma_start(out=orow[:, 0:CH], in_=ot)
                ot1 = opool.tile([P, 384], BF, tag="o1", name="ot1")
                nc.vector.tensor_scalar_mul(ot1, pso[1][:, :384], rec)
                small_dma.dma_start(out=orow[:, CH:896], in_=ot1)
                ot2 = opool.tile([P, P], BF, tag="o2", name="ot2")
                nc.vector.tensor_scalar_mul(ot2, pso[2][:, :P], rec)
                small_dma.dma_start(out=orow[:, 896:D], in_=ot2)

            def pv_psum():
                return [ps_pv.tile([P, CH], F32, tag=f"pv{ei}", name=f"pv{ei}",
                                   bufs=(2 if ei == 0 else 1)) for ei in range(3)]

            # fp8 PV for slots 0-2 (denominator fused, k-outer/chunk-inner so
            # each stationary probs tile loads once for all three chunks)
            for s in range(NSLOT - 1):
                KP = KS[s] // 2
                for qt in range(QS // P):
                    qg = 2 * s + qt                   # global 128-row q-tile id
                    pso = pv_psum()
                    for kp in range(KP):
                        for ei in (2, 0, 1):          # denom chunk first: its
                            eoff, esz = PVCH[ei]      # stop lands earliest, so
                            nc.tensor.matmul(pso[ei][:, :esz],   # rec overlaps
                                             lhsT=e8[kp][:, :, qg * P:(qg + 1) * P],
                                             rhs=v8_sb[kp][:, :, eoff:eoff + esz],
                                             start=(kp == 0), stop=(kp == KP - 1),
                                             perf_mode=DR)
                    pv_store(pso, s * QS, qt)

            # ---- slot 3: all-bf16 path (query rows 768..1023) ----
            KT = KS[NSLOT - 1]
            qr0 = (NSLOT - 1) * QS
            e_sb = [epool.tile([P, QS], BF, tag=f"e{k}", name=f"e{k}") for k in range(KT)]
            for k in range(KT):
                pss = ps_s.tile([P, QS], F32, tag="sA", name="pss")
                for d in range(DN):
                    nc.tensor.matmul(pss, lhsT=keT[d][:, k * P:(k + 1) * P],
                                     rhs=qT3[d], start=(d == 0), stop=(d == DN - 1))
                nc.scalar.activation(e_sb[k], pss, AF.Exp, scale=SCALE)
                m = mpool.tile([P, QS], BF, tag="m", name="m")
                nc.vector.tensor_scalar(m, qpos_sb[:, qr0:qr0 + QS],
                                        kpos_sb[:, k:k + 1], None, OP.is_ge)
                nc.vector.tensor_tensor(e_sb[k], e_sb[k], m, OP.mult)
            for qt in range(QS // P):
                last = qt == QS // P - 1
                pso = pv_psum()
                if last:
                    # Final q-tile: compute the denominator up front from the
                    # ones-columns alone, then run the three PV chains to
                    # completion one at a time — each output chunk scales and
                    # stores while the next chain still streams, shortening
                    # the end-of-kernel dependency tail.
                    psd = ps_s.tile([P, 16], F32, tag="sB", name="psd")
                    for k in range(KT):
                        nc.tensor.matmul(psd, lhsT=e_sb[k][:, qt * P:(qt + 1) * P],
                                         rhs=v_sb[k][:, D:DV],
                                         start=(k == 0), stop=(k == KT - 1))
                    rec = mpool.tile([P, 1], F32, tag="rec", name="rec")
                    nc.vector.reciprocal(rec, psd[:, 0:1])
                    orow = out_ext[qr0 + qt * P:qr0 + (qt + 1) * P, :]
                    for ei, (eoff, esz, osz) in enumerate([(0, CH, CH), (CH, 384, 384), (896, 144, P)]):
                        for k in range(KT):
                            nc.tensor.matmul(pso[ei][:, :esz],
                                             lhsT=e_sb[k][:, qt * P:(qt + 1) * P],
                                             rhs=v_sb[k][:, eoff:eoff + esz],
                                             start=(k == 0), stop=(k == KT - 1))
                        ot = opool.tile([P, osz], BF, tag=f"of{ei}", name=f"of{ei}")
                        if ei == 0:
                            nc.scalar.activation(ot, pso[ei][:, :osz], AF.Identity, scale=rec)
                        else:
                            nc.vector.tensor_scalar_mul(ot, pso[ei][:, :osz], rec)
                        (nc.sync if ei == 0 else nc.gpsimd).dma_start(
                            out=orow[:, eoff:eoff + osz], in_=ot)
                    continue
                for k in range(KT):
                    for ei in (2, 0, 1):
                        eoff, esz = PVCH[ei]
                        nc.tensor.matmul(pso[ei][:, :esz],
                                         lhsT=e_sb[k][:, qt * P:(qt + 1) * P],
                                         rhs=v_sb[k][:, eoff:eoff + esz],
                                         start=(k == 0), stop=(k == KT - 1))
                pv_store(pso, qr0, qt)
    if fix_waits:
        _fix_matmul_waits(nc)
    return nc


def _pair(mat):
    """[D, F] -> [D/2 rows as (dp,128p), 2, F] DoubleRow contraction pairing."""
    Dd, F = mat.shape
    return np.ascontiguousarray(
        mat.reshape(Dd // 256, 2, P, F).transpose(0, 2, 1, 3).reshape(Dd // 2, 2, F))


def _pair_blocks(mat, bs):
    """[D, F] -> [F/bs, D/2, 2, bs]: DoubleRow pairing, column-blocked so each
    [128, 2, bs] DMA tile is one contiguous DRAM region."""
    p = _pair(mat)                      # [D/2, 2, F]
    Dh, _, F = p.shape
    return np.ascontiguousarray(
        p.reshape(Dh, 2, F // bs, bs).transpose(2, 0, 1, 3))


def make_in_maps(context, W_qkv, b_qkv, n_cores=8):
    import ml_dtypes
    bf16 = ml_dtypes.bfloat16
    f8 = ml_dtypes.float8_e4m3fn
    context = np.ascontiguousarray(np.asarray(context, np.float32))
    W_qkv = np.ascontiguousarray(np.asarray(W_qkv, np.float32))
    b_qkv = np.ascontiguousarray(np.asarray(b_qkv, np.float32))
    B, N, D = context.shape
    NT = N // P
    DN = D // P
    kpos = (np.arange(NT)[None, :] * P + np.arange(P)[:, None]).astype(np.float32)
    kpos = np.ascontiguousarray(kpos)
    bq = np.ascontiguousarray(b_qkv[0:D].reshape(DN, P).T)
    bk = np.ascontiguousarray(b_qkv[D:2 * D].reshape(DN, P).T)
    bv = np.ascontiguousarray(np.broadcast_to(b_qkv[2 * D:3 * D], (P, D)))
    wq_f, wk_f, wv_f = W_qkv[:, 0:D], W_qkv[:, D:2 * D], W_qkv[:, 2 * D:3 * D]
    wk0 = np.ascontiguousarray(wk_f.astype(bf16))
    wv0 = np.ascontiguousarray(wv_f.astype(bf16))
    wq0 = np.ascontiguousarray(wq_f.astype(bf16))
    wk8 = _pair_blocks(wk_f, QS).astype(f8)
    wv8 = _pair_blocks(wv_f, CH).astype(f8)
    wq8 = _pair_blocks(wq_f, CH).astype(f8)
    in_maps = []
    for c in range(n_cores):
        b, j = divmod(c, 2)
        blocks = BLOCKS[j]
        ctx_b = context[b]
        ctxT = ctx_b.T                                   # [D, N]
        ctx0 = np.ascontiguousarray(ctxT[:, 0:QS].astype(bf16))
        ctx8 = _pair_blocks(ctxT[:, CH:], CH).astype(f8)
        ctx8b = _pair(ctxT[:, QS:CH]).astype(f8)
        rows = np.concatenate([np.arange(i * QS, (i + 1) * QS) for i in blocks])
        cqT = ctx_b[rows].T                              # [D, QTOT]
        cq8 = _pair(cqT[:, 0:QF8]).astype(f8)
        cq0 = np.ascontiguousarray(cqT[:, QF8:].astype(bf16))
        qpos_b = np.ascontiguousarray(
            np.broadcast_to(rows.astype(np.float32), (P, rows.size)))
        in_maps.append({
            "ctx0": ctx0, "ctx8": ctx8, "ctx8b": ctx8b, "cq8": cq8, "cq0": cq0,
            "wk0": wk0, "wv0": wv0, "wq0": wq0,
            "wk8": wk8, "wv8": wv8, "wq8": wq8,
            "qpos": qpos_b, "kpos": kpos, "bqT": bq, "bkT": bk, "bvb": bv,
        })
    return in_maps


def assemble(results, B, N, D):
    out = np.zeros((B, N, D), np.float32)
    for c, res in enumerate(results):
        b, j = divmod(c, 2)
        o = np.asarray(res["out"], np.float32)
        for s, i in enumerate(BLOCKS[j]):
            out[b, i * QS:(i + 1) * QS] = o[s * QS:(s + 1) * QS]
    return out


def run(inputs, trace=False, **spmd_kwargs):
    context = np.asarray(inputs["context"])
    B, N, D = context.shape
    nc = build(N, D)
    in_maps = make_in_maps(context, inputs["W_qkv"], inputs["b_qkv"], n_cores=8)
    res = run_bass_kernel_spmd(nc, in_maps, core_ids=list(range(8)), trace=trace, **spmd_kwargs)
    out = assemble(res.results, B, N, D)
    return out, res


def kernel(context, W_qkv, b_qkv):
    out, _ = run({"context": context, "W_qkv": W_qkv, "b_qkv": b_qkv})
    return out


# revision 4
# speedup vs baseline: 1.0768x; 1.0026x over previous
"""Causal self-attention (QKV projection + softmax(QK^T/sqrt(N)) @ V) on 8 TRN2
NeuronCores — fp8 DoubleRow edition.

Sharding as the bf16 ancestor: core c = 2*b + j handles batch element b and
half its query rows as four 256-row blocks, interleaved for causal load
balance: j=0 -> [7,4,3,0], j=1 -> [6,5,2,1]; SPMD slot k-tile counts
KS=[16,12,8,4] with position-mask cleanup on the last 4 k-tiles per slot.

fp8 (e4m3) DoubleRow doubles PE throughput (2 contraction elements per cell
per cycle) on the projection matmuls.  Numerics (validated offline, rel-err
7.1e-3 vs 2e-2 budget): quantization noise in q/k/v is harmless for query
rows >= 512 (softmax averaging washes it out: error ~ eps*sqrt(sum w^2)), but
early rows expose single V rows and logit noise directly.  So keys 0-511 and
query rows 0-511 (slot 3 + K/V chunk 0) stay on the bf16 path; keys/queries
512+ use fp8 DoubleRow projections.  Projection OUTPUTS stay bf16 (scores and
PV are bf16 here).

DoubleRow operand layout: [128 part, 2, F] — dim 1 selects the contraction
pair member (d-tile 2*dp / 2*dp+1), host pre-pairs both W and ctx.
"""

import math
from contextlib import ExitStack

import numpy as np

import concourse.bass as bass
import concourse.mybir as mybir
import concourse.tile as tile
from concourse.bass_utils import run_bass_kernel_spmd
from concourse.tile_rust import add_dep_helper

P = 128
CH = 512          # free-dim chunk for projection matmuls (one PSUM bank, fp32)
QS = 256          # query rows per attention slot
KS = [16, 12, 8, 4]          # k-tiles per slot (uniform SPMD schedule)
MASK_TILES = 4               # last 4 k-tiles of every slot get masked
BLOCKS = ([7, 4, 3, 0], [6, 5, 2, 1])   # 256-row block ids per core parity
QF8 = 768         # query rows on the fp8 path (slots 0-2)


def _chunks(total, size):
    return [(o, min(size, total - o)) for o in range(0, total, size)]


def _fix_matmul_waits(nc):
    """Walrus codegen has a small per-instruction sync-wait slot budget (one
    for a self-loading matmul's LDWEIGHTS half, similar for ACT etc).  Move
    extra waits onto NoOps inserted just before the instruction on the same
    engine -- per-engine program order (and thus semantics) is unchanged."""
    skip = (mybir.InstEventSemaphore, mybir.InstNoOp,
            mybir.InstUnconditionalBranch, mybir.InstCall)
    for func in nc.m.functions:
        for bb in func.blocks:
            il = bb.instructions
            new = []
            changed = False
            for inst in il:
                si = getattr(inst, "sync_info", None)
                if (si and si.on_wait and len(si.on_wait) > 1
                        and not isinstance(inst, skip)):
                    waits = list(si.on_wait)
                    for wi, w in enumerate(waits[:-1]):
                        nop = mybir.InstNoOp(
                            name=f"{inst.name}-wfix{wi}", engine=inst.engine,
                            sync_info=mybir.SyncInfo(on_wait=[w], on_update=[]),
                            text_hint="waitfix")
                        new.append(nop)
                    inst.sync_info = mybir.SyncInfo(
                        on_wait=[waits[-1]], on_update=list(si.on_update or []))
                    changed = True
                new.append(inst)
            if changed:
                bb.instructions = new


def build(N=2048, D=1024, fix_waits=True, **bass_kwargs):
    NT = N // P          # number of 128-row key tiles (16)
    DN = D // P          # contraction tiles / e-tiles (8)
    DP = DN // 2         # contraction pair-tiles for DoubleRow (4)
    QTOT = N // 2        # query rows per core (1024)
    NSLOT = QTOT // QS   # attention slots (4)
    SCALE = 1.0 / math.sqrt(N)
    BF = mybir.dt.bfloat16
    F8 = mybir.dt.float8e4
    F32 = mybir.dt.float32
    AF = mybir.ActivationFunctionType
    OP = mybir.AluOpType
    DR = mybir.MatmulPerfMode.DoubleRow

    nc = bass.Bass(**bass_kwargs)
    anchors = []  # first K-proj matmul of each ctx chunk; DMA stage gates

    def _dep(dma_bi, on):
        add_dep_helper(dma_bi.ins, on.ins, sync=True, reason="dma staging")
        return dma_bi

    def _after(dma_bi, anchor_idx):
        if anchor_idx < len(anchors):
            _dep(dma_bi, anchors[anchor_idx])
        return dma_bi

    # bf16 operands (keys 0-511 / slot-3 queries)
    ctx0 = nc.declare_dram_parameter("ctx0", [D, QS], BF, isOutput=False)
    cq0 = nc.declare_dram_parameter("cq0", [D, QS], BF, isOutput=False)
    wk0 = nc.declare_dram_parameter("wk0", [D, D], BF, isOutput=False)
    wv0 = nc.declare_dram_parameter("wv0", [D, D], BF, isOutput=False)
    wq0 = nc.declare_dram_parameter("wq0", [D, D], BF, isOutput=False)
    # fp8 DoubleRow operands, contraction-paired [.., dp*128+p, i, :].
    # Weights are split into column-half blocks and ctx into key-chunk blocks
    # so every DMA tile is a contiguous DRAM region (1KB+ rows -> full DMA
    # descriptor efficiency; a flat pair layout would leave 512B strided
    # segments and halve effective HBM bandwidth).
    ctx8 = nc.declare_dram_parameter("ctx8", [NT * P // CH - 1, DP * P, 2, CH], F8, isOutput=False)
    ctx8b = nc.declare_dram_parameter("ctx8b", [DP * P, 2, QS], F8, isOutput=False)
    cq8 = nc.declare_dram_parameter("cq8", [DP * P, 2, QF8], F8, isOutput=False)
    wk8 = nc.declare_dram_parameter("wk8", [4, DP * P, 2, QS], F8, isOutput=False)
    wv8 = nc.declare_dram_parameter("wv8", [2, DP * P, 2, CH], F8, isOutput=False)
    wq8 = nc.declare_dram_parameter("wq8", [2, DP * P, 2, CH], F8, isOutput=False)

    qpos = nc.declare_dram_parameter("qpos", [P, QTOT], F32, isOutput=False)
    kpos = nc.declare_dram_parameter("kpos", [P, NT], F32, isOutput=False)
    bqT = nc.declare_dram_parameter("bqT", [P, DN], F32, isOutput=False)
    bkT = nc.declare_dram_parameter("bkT", [P, DN], F32, isOutput=False)
    bvb = nc.declare_dram_parameter("bvb", [P, D], F32, isOutput=False)
    out_ext = nc.declare_dram_parameter("out", [QTOT, D], BF, isOutput=True)

    with ExitStack() as ctx:
        tc = ctx.enter_context(tile.TileContext(nc))
        const = ctx.enter_context(tc.tile_pool(name="const", bufs=1))
        persist = ctx.enter_context(tc.tile_pool(name="persist", bufs=1))

        # PE warmup: dummy matmuls on a memset tile bridge the DMA-dead
        # startup window so the HAM clock gate is at 8/8 when real data lands.
        with tc.tile_pool(name="warm", bufs=1) as warmp, \
             tc.tile_pool(name="warmps", bufs=4, space="PSUM") as warmpp:
            wtile = warmp.tile([P, 3 * P], BF)
            nc.vector.memset(wtile, 0.0)
            for _ in range(30):
                wps = warmpp.tile([P, 3 * P], F32, tag="w", name="wps")
                nc.tensor.matmul(wps, lhsT=wtile[:, 0:P], rhs=wtile, start=True, stop=True)

        bq_sb = const.tile([P, DN], F32)
        nc.sync.dma_start(out=bq_sb, in_=bqT[:, :])
        bk_sb = const.tile([P, DN], F32)
        nc.sync.dma_start(out=bk_sb, in_=bkT[:, :])
        kpos_sb = const.tile([P, NT], F32)
        nc.sync.dma_start(out=kpos_sb, in_=kpos[:, :])
        qpos_sb = const.tile([P, QTOT], F32)
        bv_sb = const.tile([P, D], F32)

        # K^T and V both SBUF-resident for the whole kernel.  V tiles carry 16
        # extra ones-columns: the PV matmul then accumulates the softmax
        # denominator for free in its last output chunk (no separate
        # denominator matmuls).  v8 = fp8 contraction-paired V (all 16 k-tiles,
        # for the fp8 DoubleRow PV of slots 0-2); v_sb = bf16 V (k-tiles 0-3,
        # for slot 3's bf16 PV).
        DV = D + 16
        # K^T fp8 e-paired (all 16 k-tiles, for fp8 scores of slots 0-2) +
        # bf16 K^T for k-tiles 0-3 only (slot 3's bf16 scores).
        keT8 = [persist.tile([P, 2, N], F8, tag=f"ke8{ep}", name=f"ke8{ep}") for ep in range(DN // 2)]
        keT = [persist.tile([P, CH], BF, tag=f"ke{e}", name=f"ke{e}") for e in range(DN)]
        v_sb = [persist.tile([P, DV], BF, tag=f"v{t}", name=f"v{t}") for t in range(4)]
        v8_sb = [persist.tile([P, 2, DV], F8, tag=f"v8{tp}", name=f"v8{tp}") for tp in range(NT // 2)]
        for t in range(4):
            nc.vector.memset(v_sb[t][:, D:DV], 1.0)
        for tp in range(NT // 2):
            nc.vector.memset(v8_sb[tp][:, 0, D:DV], 1.0)
            nc.vector.memset(v8_sb[tp][:, 1, D:DV], 1.0)

        # Q-phase tiles live in outer pools so their DMAs can be issued early.
        wqp = ctx.enter_context(tc.tile_pool(name="wq", bufs=1))
        ctxq = ctx.enter_context(tc.tile_pool(name="ctxq", bufs=1))
        wq8_sb = [[wqp.tile([P, 2, CH], F8, tag=f"wq8{h}{dp}", name=f"wq8{h}{dp}")
                   for dp in range(DP)] for h in range(2)]
        wq0_sb = [wqp.tile([P, D], BF, tag=f"wq0{d}", name=f"wq0{d}") for d in range(DN)]
        cq8_sb = [ctxq.tile([P, 2, QF8], F8, tag=f"cq8{dp}", name=f"cq8{dp}") for dp in range(DP)]
        cq0_sb = [ctxq.tile([P, QS], BF, tag=f"cq0{d}", name=f"cq0{d}") for d in range(DN)]

        # ---------------- K/V projection ----------------
        # ALL K chunks first (fp8 keys 512.. then bf16 keys 0-511), THEN all V
        # chunks: every V operand deadline moves ~40us later, so the startup
        # DMA stream only has to race the K weights.  wk8 ships in quarter
        # column-blocks so the first matmul waits for just 0.25MB + ctx.
        with tc.tile_pool(name="wkv", bufs=1) as wkv, \
             tc.tile_pool(name="ctxp", bufs=1) as ctxp, \
             tc.tile_pool(name="pp", bufs=5, space="PSUM") as pp:
            wk8_sb = [[wkv.tile([P, 2, QS], F8, tag=f"wk8{qq}{dp}", name=f"wk8{qq}{dp}")
                       for dp in range(DP)] for qq in range(4)]
            wv8_sb = [[wkv.tile([P, 2, CH], F8, tag=f"wv8{h}{dp}", name=f"wv8{h}{dp}")
                       for dp in range(DP)] for h in range(2)]
            wk0_sb = [wkv.tile([P, D], BF, tag=f"wk0{d}", name=f"wk0{d}") for d in range(DN)]
            wv0_sb = [wkv.tile([P, D], BF, tag=f"wv0{d}", name=f"wv0{d}") for d in range(DN)]

            HF = D // 2
            # startup-critical: wk8 quarter 0 (sync ring) + ctx8 chunk 1
            # (gpsimd ring) in parallel; later batches issue in parallel
            # within a batch, chained batch-to-batch + compute-anchor gates.
            # Within a ring, program order IS execution order — no completion
            # deps needed (they only add a latency bubble per batch).  Anchor
            # gates alone hold lower-priority transfers back.
            cts8 = {}
            for dp in range(DP):
                ct = ctxp.tile([P, 2, CH], F8, tag=f"c8_1_{dp}", name=f"c8_1_{dp}")
                nc.gpsimd.dma_start(out=ct, in_=ctx8[0, dp * P:(dp + 1) * P, :, :])
                cts8[(1, dp)] = ct
            for qq in range(4):
                for dp in range(DP):
                    nc.sync.dma_start(out=wk8_sb[qq][dp], in_=wk8[qq, dp * P:(dp + 1) * P, :, :])
            nc.sync.dma_start(out=bv_sb, in_=bvb[:, :])

            def k_chunk_fp8(ci, merge_b=False):
                # merge_b: fold keys 256-511 in as a second moving group that
                # reuses each stationary load (alone they'd be LDW-bound).
                coff = ci * CH
                cts = [cts8[(ci, dp)] for dp in range(DP)]
                cb8 = [cts8[("b", dp)] for dp in range(DP)] if merge_b else None
                for e in range(DN):
                    psk = pp.tile([P, CH], F32, tag="pp8", name="psk")
                    psB = pp.tile([P, QS], F32, tag="ppB", name="psB", bufs=3) if merge_b else None
                    for dp in range(DP):
                        lhs = wk8_sb[e // 2][dp][:, :, (e % 2) * P:(e % 2 + 1) * P]
                        mm = nc.tensor.matmul(psk, lhsT=lhs, rhs=cts[dp],
                                              start=(dp == 0), stop=(dp == DP - 1),
                                              perf_mode=DR)
                        if e == 0 and dp == 0:
                            anchors.append(mm)
                        if merge_b:
                            nc.tensor.matmul(psB, lhsT=lhs, rhs=cb8[dp],
                                             start=(dp == 0), stop=(dp == DP - 1),
                                             perf_mode=DR)
                    nc.scalar.activation(keT8[e // 2][:, e % 2, coff:coff + CH], psk,
                                         AF.Identity, bias=bk_sb[:, e:e + 1], scale=1.0)
                    if merge_b:
                        nc.scalar.activation(keT8[e // 2][:, e % 2, QS:CH], psB,
                                             AF.Identity, bias=bk_sb[:, e:e + 1], scale=1.0)
                        nc.vector.tensor_scalar(keT[e][:, QS:CH], psB,
                                                bk_sb[:, e:e + 1], None, OP.add)

            def k_chunk_bf16():
                cts = [cts8[(0, d)] for d in range(DN)]
                for e in range(DN):
                    psk = pp.tile([P, QS], F32, tag="ppB", name="psk0", bufs=3)
                    for d in range(DN):
                        mm = nc.tensor.matmul(psk, lhsT=wk0_sb[d][:, e * P:(e + 1) * P],
                                              rhs=cts[d], start=(d == 0), stop=(d == DN - 1))
                        if e == 0 and d == 0:
                            anchors.append(mm)
                    nc.scalar.activation(keT[e][:, 0:QS], psk,
                                         AF.Identity, bias=bk_sb[:, e:e + 1], scale=1.0)
                    nc.vector.tensor_scalar(keT8[e // 2][:, e % 2, 0:QS], psk,
                                            bk_sb[:, e:e + 1], None, OP.add)

            def v_chunk_fp8(ci):
                coff = ci * CH
                cts = [cts8[(ci, dp)] for dp in range(DP)]
                for eoff, esz in _chunks(D, CH):
                    for ntl in range(CH // P):
                        n_t = coff // P + ntl
                        psv = pp.tile([P, CH], F32, tag="pp8", name="psv")
                        for dp in range(DP):
                            nc.tensor.matmul(psv[:, :esz], lhsT=cts[dp][:, :, ntl * P:(ntl + 1) * P],
                                             rhs=wv8_sb[eoff // CH][dp],
                                             start=(dp == 0), stop=(dp == DP - 1), perf_mode=DR)
                        nc.vector.tensor_tensor(v8_sb[n_t // 2][:, n_t % 2, eoff:eoff + esz],
                                                psv[:, :esz], bv_sb[:, eoff:eoff + esz], OP.add)

            def v_chunk_bf16():
                # V keys 0-255 bf16, keys 256-511 fp8 DoubleRow (v-noise only
                # reaches rows >= 256, where softmax averaging absorbs it).
                # Both halves dual-write the bf16 copy (slot 3's PV) and the
                # fp8 paired copy (slots 0-2).
                cts = [cts8[(0, d)] for d in range(DN)]
                cb8 = [cts8[("b", dp)] for dp in range(DP)]
                for eoff, esz in _chunks(D, CH):
                    for ntl in range(QS // P):
                        psv = pp.tile([P, CH], F32, tag="pp8", name="psv")
                        for d in range(DN):
                            nc.tensor.matmul(psv[:, :esz], lhsT=cts[d][:, ntl * P:(ntl + 1) * P],
                                             rhs=wv0_sb[d][:, eoff:eoff + esz],
                                             start=(d == 0), stop=(d == DN - 1))
                        nc.vector.tensor_tensor(v_sb[ntl][:, eoff:eoff + esz], psv[:, :esz],
                                                bv_sb[:, eoff:eoff + esz], OP.add)
                        # duplicate fp8 copy from the finished bf16 tile on the
                        # (idle) scalar engine — a second vector read of psum
                        # would make this phase DVE-bound
                        nc.scalar.activation(v8_sb[0][:, ntl, eoff:eoff + esz],
                                             v_sb[ntl][:, eoff:eoff + esz], AF.Identity, scale=1.0)
                    for ntl in range(QS // P):
                        psv = pp.tile([P, CH], F32, tag="pp8", name="psv")
                        for dp in range(DP):
                            nc.tensor.matmul(psv[:, :esz], lhsT=cb8[dp][:, :, ntl * P:(ntl + 1) * P],
                                             rhs=wv8_sb[eoff // CH][dp],
                                             start=(dp == 0), stop=(dp == DP - 1), perf_mode=DR)
                        nc.vector.tensor_tensor(v_sb[2 + ntl][:, eoff:eoff + esz], psv[:, :esz],
                                                bv_sb[:, eoff:eoff + esz], OP.add)
                        nc.scalar.activation(v8_sb[1][:, ntl, eoff:eoff + esz],
                                             v_sb[2 + ntl][:, eoff:eoff + esz], AF.Identity, scale=1.0)

            for ci in (1, 2, 3):
                if ci > 1:      # ctx8 chunk ci: plain ring order behind c1
                    for dp in range(DP):
                        ct = ctxp.tile([P, 2, CH], F8, tag=f"c8_{ci}_{dp}", name=f"c8_{ci}_{dp}")
                        nc.gpsimd.dma_start(out=ct, in_=ctx8[ci - 1, dp * P:(dp + 1) * P, :, :])
                        cts8[(ci, dp)] = ct
                if ci == 2:     # keys 256-511 (K3 B-group + V0), bf16 K operands
                    for dp in range(DP):
                        ct = ctxp.tile([P, 2, QS], F8, tag=f"c8b_{dp}", name=f"c8b_{dp}")
                        _after(nc.sync.dma_start(out=ct, in_=ctx8b[dp * P:(dp + 1) * P, :, :]), 0)
                        cts8[("b", dp)] = ct
                    for d in range(DN):
                        _after(nc.sync.dma_start(
                            out=wk0_sb[d][:, 0:HF], in_=wk0[d * P:(d + 1) * P, 0:HF]), 1)
                    for d in range(DN):
                        ct = ctxp.tile([P, QS], BF, tag=f"c0_{d}", name=f"c0_{d}")
                        _after(nc.gpsimd.dma_start(out=ct, in_=ctx0[d * P:(d + 1) * P, :]), 1)
                        cts8[(0, d)] = ct
                    for d in range(DN):
                        _after(nc.sync.dma_start(
                            out=wk0_sb[d][:, HF:D], in_=wk0[d * P:(d + 1) * P, HF:D]), 2)
                k_chunk_fp8(ci, merge_b=(ci == 3))
            k_chunk_bf16()
            # V-phase + Q-phase operands, all gated well behind their use
            for h in range(2):
                for d in range(DN):
                    _after(nc.sync.dma_start(
                        out=wv0_sb[d][:, h * HF:(h + 1) * HF],
                        in_=wv0[d * P:(d + 1) * P, h * HF:(h + 1) * HF]), 2)
            for h in range(2):
                for dp in range(DP):
                    _after(nc.sync.dma_start(
                        out=wv8_sb[h][dp], in_=wv8[h, dp * P:(dp + 1) * P, :, :]), 2)
            v_chunk_bf16()
            for h in range(2):
                for dp in range(DP):
                    _after(nc.sync.dma_start(
                        out=wq8_sb[h][dp], in_=wq8[h, dp * P:(dp + 1) * P, :, :]), 3)
            for dp in range(DP):
                _after(nc.sync.dma_start(out=cq8_sb[dp], in_=cq8[dp * P:(dp + 1) * P, :, :]), 3)
            for ci in (1, 2, 3):
                v_chunk_fp8(ci)
            for d in range(DN):
                _after(nc.sync.dma_start(out=wq0_sb[d], in_=wq0[d * P:(d + 1) * P, :]), 3)
            for d in range(DN):
                _after(nc.sync.dma_start(out=cq0_sb[d], in_=cq0[d * P:(d + 1) * P, :]), 3)
            _after(nc.sync.dma_start(out=qpos_sb, in_=qpos[:, :]), 3)

        # ---------------- Q projection + attention slots ----------------
        with tc.tile_pool(name="qtb", bufs=1) as qtb, \
             tc.tile_pool(name="att_e", bufs=2) as epool, \
             tc.tile_pool(name="att_m", bufs=3) as mpool, \
             tc.tile_pool(name="att_o", bufs=3) as opool, \
             tc.tile_pool(name="ps_s", bufs=2, space="PSUM") as ps_s, \
             tc.tile_pool(name="ps_pv", bufs=2, space="PSUM") as ps_pv:
            # Q^T: fp8 e-paired for rows 0..767 (slots 0-2), bf16 for slot 3.
            qT8 = [qtb.tile([P, 2, QF8], F8, tag=f"qt8{ep}", name=f"qt8{ep}") for ep in range(DN // 2)]
            qT3 = [qtb.tile([P, QS], BF, tag=f"qt3{e}", name=f"qt3{e}") for e in range(DN)]
            # fp8 rows (slots 0-2): both query chunks (cols 0:512 and 512:768)
            # share each stationary load — the 256-col chunk alone would be
            # LDWEIGHTS-bound.
            for e in range(DN):
                psqA = ps_s.tile([P, CH], F32, tag="sA", name="psqA")
                psqB = ps_s.tile([P, QS], F32, tag="sB", name="psqB")
                for dp in range(DP):
                    lhs = wq8_sb[e // 4][dp][:, :, (e % 4) * P:(e % 4 + 1) * P]
                    nc.tensor.matmul(psqA, lhsT=lhs, rhs=cq8_sb[dp][:, :, 0:CH],
                                     start=(dp == 0), stop=(dp == DP - 1), perf_mode=DR)
                    nc.tensor.matmul(psqB, lhsT=lhs, rhs=cq8_sb[dp][:, :, CH:QF8],
                                     start=(dp == 0), stop=(dp == DP - 1), perf_mode=DR)
                nc.scalar.activation(qT8[e // 2][:, e % 2, 0:CH], psqA,
                                     AF.Identity, bias=bq_sb[:, e:e + 1], scale=1.0)
                nc.scalar.activation(qT8[e // 2][:, e % 2, CH:QF8], psqB,
                                     AF.Identity, bias=bq_sb[:, e:e + 1], scale=1.0)
            for e in range(DN):                       # bf16 rows (slot 3)
                psq = ps_s.tile([P, CH], F32, tag="sA", name="psq")
                for d in range(DN):
                    nc.tensor.matmul(psq[:, :QS], lhsT=wq0_sb[d][:, e * P:(e + 1) * P],
                                     rhs=cq0_sb[d], start=(d == 0), stop=(d == DN - 1))
                nc.scalar.activation(qT3[e], psq[:, :QS],
                                     AF.Identity, bias=bq_sb[:, e:e + 1], scale=1.0)

            # ---- fp8 scores for slots 0-2, k-merged ----
            # Slots 0-2 occupy contiguous query columns 0..767, so one pass
            # per k-tile serves every slot that needs it (group A = cols
            # 0..511 -> slots 0+1, group B = cols 512..767 -> slot 2); the
            # DoubleRow stationary (256-col LDWEIGHTS) amortizes across all
            # live query columns instead of being reloaded per slot.
            e8 = [epool.tile([P, 2, QF8], F8, tag=f"e8_{kp}", name=f"e8_{kp}")
                  for kp in range(KS[0] // 2)]

            def mask_e8(k, qc0):
                m = mpool.tile([P, QS], BF, tag="m", name="m")
                nc.vector.tensor_scalar(m, qpos_sb[:, qc0:qc0 + QS],
                                        kpos_sb[:, k:k + 1], None, OP.is_ge)
                nc.vector.tensor_tensor(e8[k // 2][:, k % 2, qc0:qc0 + QS],
                                        e8[k // 2][:, k % 2, qc0:qc0 + QS], m, OP.mult)

            for k in range(KS[0]):
                qnA = CH if k < KS[1] else QS         # slots 0+1, or 0 only
                pssA = ps_s.tile([P, CH], F32, tag="sA", name="pssA")
                pssB = ps_s.tile([P, QS], F32, tag="sB", name="pssB") if k < KS[2] else None
                for ep in range(DN // 2):
                    lhs = keT8[ep][:, :, k * P:(k + 1) * P]
                    nc.tensor.matmul(pssA[:, :qnA], lhsT=lhs, rhs=qT8[ep][:, :, 0:qnA],
                                     start=(ep == 0), stop=(ep == DN // 2 - 1), perf_mode=DR)
                    if pssB is not None:
                        nc.tensor.matmul(pssB, lhsT=lhs, rhs=qT8[ep][:, :, CH:QF8],
                                         start=(ep == 0), stop=(ep == DN // 2 - 1), perf_mode=DR)
                nc.scalar.activation(e8[k // 2][:, k % 2, 0:qnA], pssA[:, :qnA], AF.Exp, scale=SCALE)
                if pssB is not None:
                    nc.scalar.activation(e8[k // 2][:, k % 2, CH:QF8], pssB, AF.Exp, scale=SCALE)
                if k >= KS[2]:                        # mask tail k-tiles per slot
                    s_of_k = 0 if k >= KS[1] else 1
                    mask_e8(k, s_of_k * QS)
                elif k >= KS[2] - MASK_TILES:
                    mask_e8(k, 2 * QS)

            # PV output chunks: (512, 384, 144) — the 144-chunk covers out
            # cols 896..1023 plus the 16 ones-columns, whose accumulated sums
            # ARE the softmax denominator (psum cols 128..143).
            PVCH = [(0, CH), (CH, 384), (896, 144)]

            def pv_store(pso, qr0, qt, last=False):
                # On the final q-tile, send the big chunk down the sync ring
                # (the gpsimd ring then drains earlier, off the critical tail).
                big_dma, small_dma = (nc.sync, nc.gpsimd) if last else (nc.gpsimd, nc.sync)
                rec = mpool.tile([P, 1], F32, tag="rec", name="rec")
                nc.vector.reciprocal(rec, pso[2][:, P:P + 1])
                orow = out_ext[qr0 + qt * P:qr0 + (qt + 1) * P, :]
                ot = opool.tile([P, CH], BF, tag="o", name="ot")
                nc.scalar.activation(ot, pso[0], AF.Identity, scale=rec)
                big_dma.dma_start(out=orow[:, 0:CH], in_=ot)
                ot1 = opool.tile([P, 384], BF, tag="o1", name="ot1")
                nc.vector.tensor_scalar_mul(ot1, pso[1][:, :384], rec)
                small_dma.dma_start(out=orow[:, CH:896], in_=ot1)
                ot2 = opool.tile([P, P], BF, tag="o2", name="ot2")
                nc.vector.tensor_scalar_mul(ot2, pso[2][:, :P], rec)
                small_dma.dma_start(out=orow[:, 896:D], in_=ot2)

            def pv_psum():
                return [ps_pv.tile([P, CH], F32, tag=f"pv{ei}", name=f"pv{ei}",
                                   bufs=(2 if ei == 0 else 1)) for ei in range(3)]

            # fp8 PV for slots 0-2 (denominator fused, k-outer/chunk-inner so
            # each stationary probs tile loads once for all three chunks)
            for s in range(NSLOT - 1):
                KP = KS[s] // 2
                for qt in range(QS // P):
                    qg = 2 * s + qt                   # global 128-row q-tile id
                    pso = pv_psum()
                    for kp in range(KP):
                        for ei in (2, 0, 1):          # denom chunk first: its
                            eoff, esz = PVCH[ei]      # stop lands earliest, so
                            nc.tensor.matmul(pso[ei][:, :esz],   # rec overlaps
                                             lhsT=e8[kp][:, :, qg * P:(qg + 1) * P],
                                             rhs=v8_sb[kp][:, :, eoff:eoff + esz],
                                             start=(kp == 0), stop=(kp == KP - 1),
                                             perf_mode=DR)
                    pv_store(pso, s * QS, qt)

            # ---- slot 3: all-bf16 path (query rows 768..1023) ----
            KT = KS[NSLOT - 1]
            qr0 = (NSLOT - 1) * QS
            e_sb = [epool.tile([P, QS], BF, tag=f"e{k}", name=f"e{k}") for k in range(KT)]
            for k in range(KT):
                pss = ps_s.tile([P, QS], F32, tag="sA", name="pss")
                for d in range(DN):
                    nc.tensor.matmul(pss, lhsT=keT[d][:, k * P:(k + 1) * P],
                                     rhs=qT3[d], start=(d == 0), stop=(d == DN - 1))
                nc.scalar.activation(e_sb[k], pss, AF.Exp, scale=SCALE)
                m = mpool.tile([P, QS], BF, tag="m", name="m")
                nc.vector.tensor_scalar(m, qpos_sb[:, qr0:qr0 + QS],
                                        kpos_sb[:, k:k + 1], None, OP.is_ge)
                nc.vector.tensor_tensor(e_sb[k], e_sb[k], m, OP.mult)
            for qt in range(QS // P):
                last = qt == QS // P - 1
                pso = pv_psum()
                if last:
                    # Final q-tile: compute the denominator up front from the
                    # ones-columns alone, then run the three PV chains to
                    # completion one at a time — each output chunk scales and
                    # stores while the next chain still streams, shortening
                    # the end-of-kernel dependency tail.
                    psd = ps_s.tile([P, 16], F32, tag="sB", name="psd")
                    for k in range(KT):
                        nc.tensor.matmul(psd, lhsT=e_sb[k][:, qt * P:(qt + 1) * P],
                                         rhs=v_sb[k][:, D:DV],
                                         start=(k == 0), stop=(k == KT - 1))
                    rec = mpool.tile([P, 1], F32, tag="rec", name="rec")
                    nc.vector.reciprocal(rec, psd[:, 0:1])
                    orow = out_ext[qr0 + qt * P:qr0 + (qt + 1) * P, :]
                    for ei, (eoff, esz, osz) in enumerate([(0, CH, CH), (CH, 384, 384), (896, 144, P)]):
                        for k in range(KT):
                            nc.tensor.matmul(pso[ei][:, :esz],
                                             lhsT=e_sb[k][:, qt * P:(qt + 1) * P],
                                             rhs=v_sb[k][:, eoff:eoff + esz],
                                             start=(k == 0), stop=(k == KT - 1))
                        ot = opool.tile([P, osz], BF, tag=f"of{ei}", name=f"of{ei}")
                        if ei == 0:
                            nc.scalar.activation(ot, pso[ei][:, :osz], AF.Identity, scale=rec)
                        else:
                            nc.vector.tensor_scalar_mul(ot, pso[ei][:, :osz], rec)
                        (nc.sync if ei == 0 else nc.gpsimd).dma_start(
                            out=orow[:, eoff:eoff + osz], in_=ot)
                    continue
                for k in range(KT):
                    for ei in (2, 0, 1):
                        eoff, esz = PVCH[ei]
                        nc.tensor.matmul(pso[ei][:, :esz],
                                         lhsT=e_sb[k][:, qt * P:(qt + 1) * P],
                                         rhs=v_sb[k][:, eoff:eoff + esz],
                                         start=(k == 0), stop=(k == KT - 1))
                pv_store(pso, qr0, qt)
    if fix_waits:
        _fix_matmul_waits(nc)
    return nc


def _pair(mat):
    """[D, F] -> [D/2 rows as (dp,128p), 2, F] DoubleRow contraction pairing."""
    Dd, F = mat.shape
    return np.ascontiguousarray(
        mat.reshape(Dd // 256, 2, P, F).transpose(0, 2, 1, 3).reshape(Dd // 2, 2, F))


def _pair_blocks(mat, bs):
    """[D, F] -> [F/bs, D/2, 2, bs]: DoubleRow pairing, column-blocked so each
    [128, 2, bs] DMA tile is one contiguous DRAM region."""
    p = _pair(mat)                      # [D/2, 2, F]
    Dh, _, F = p.shape
    return np.ascontiguousarray(
        p.reshape(Dh, 2, F // bs, bs).transpose(2, 0, 1, 3))


def make_in_maps(context, W_qkv, b_qkv, n_cores=8):
    import ml_dtypes
    bf16 = ml_dtypes.bfloat16
    f8 = ml_dtypes.float8_e4m3fn
    context = np.ascontiguousarray(np.asarray(context, np.float32))
    W_qkv = np.ascontiguousarray(np.asarray(W_qkv, np.float32))
    b_qkv = np.ascontiguousarray(np.asarray(b_qkv, np.float32))
    B, N, D = context.shape
    NT = N // P
    DN = D // P
    kpos = (np.arange(NT)[None, :] * P + np.arange(P)[:, None]).astype(np.float32)
    kpos = np.ascontiguousarray(kpos)
    bq = np.ascontiguousarray(b_qkv[0:D].reshape(DN, P).T)
    bk = np.ascontiguousarray(b_qkv[D:2 * D].reshape(DN, P).T)
    bv = np.ascontiguousarray(np.broadcast_to(b_qkv[2 * D:3 * D], (P, D)))
    wq_f, wk_f, wv_f = W_qkv[:, 0:D], W_qkv[:, D:2 * D], W_qkv[:, 2 * D:3 * D]
    wk0 = np.ascontiguousarray(wk_f.astype(bf16))
    wv0 = np.ascontiguousarray(wv_f.astype(bf16))
    wq0 = np.ascontiguousarray(wq_f.astype(bf16))
    wk8 = _pair_blocks(wk_f, QS).astype(f8)
    wv8 = _pair_blocks(wv_f, CH).astype(f8)
    wq8 = _pair_blocks(wq_f, CH).astype(f8)
    in_maps = []
    for c in range(n_cores):
        b, j = divmod(c, 2)
        blocks = BLOCKS[j]
        ctx_b = context[b]
        ctxT = ctx_b.T                                   # [D, N]
        ctx0 = np.ascontiguousarray(ctxT[:, 0:QS].astype(bf16))
        ctx8 = _pair_blocks(ctxT[:, CH:], CH).astype(f8)
        ctx8b = _pair(ctxT[:, QS:CH]).astype(f8)
        rows = np.concatenate([np.arange(i * QS, (i + 1) * QS) for i in blocks])
        cqT = ctx_b[rows].T                              # [D, QTOT]
        cq8 = _pair(cqT[:, 0:QF8]).astype(f8)
        cq0 = np.ascontiguousarray(cqT[:, QF8:].astype(bf16))
        qpos_b = np.ascontiguousarray(
            np.broadcast_to(rows.astype(np.float32), (P, rows.size)))
        in_maps.append({
            "ctx0": ctx0, "ctx8": ctx8, "ctx8b": ctx8b, "cq8": cq8, "cq0": cq0,
            "wk0": wk0, "wv0": wv0, "wq0": wq0,
            "wk8": wk8, "wv8": wv8, "wq8": wq8,
            "qpos": qpos_b, "kpos": kpos, "bqT": bq, "bkT": bk, "bvb": bv,
        })
    return in_maps


def assemble(results, B, N, D):
    out = np.zeros((B, N, D), np.float32)
    for c, res in enumerate(results):
        b, j = divmod(c, 2)
        o = np.asarray(res["out"], np.float32)
        for s, i in enumerate(BLOCKS[j]):
            out[b, i * QS:(i + 1) * QS] = o[s * QS:(s + 1) * QS]
    return out


def run(inputs, trace=False, **spmd_kwargs):
    context = np.asarray(inputs["context"])
    B, N, D = context.shape
    nc = build(N, D)
    in_maps = make_in_maps(context, inputs["W_qkv"], inputs["b_qkv"], n_cores=8)
    res = run_bass_kernel_spmd(nc, in_maps, core_ids=list(range(8)), trace=trace, **spmd_kwargs)
    out = assemble(res.results, B, N, D)
    return out, res


def kernel(context, W_qkv, b_qkv):
    out, _ = run({"context": context, "W_qkv": W_qkv, "b_qkv": b_qkv})
    return out
